# revision 20
# baseline (speedup 1.0000x reference)
"""Trainium2 Bass kernel for nn_AvaAttention (GQA attention, head-constant RoPE).

Sharding: tensor-parallel over the 8 kv heads -> core c owns kv head c and
q heads 4c..4c+3. Each core computes its 4 heads' attention and a partial
o_proj (row-split Wo); the host sums the 8 partials.

Key device-side design points (v2, rates from on-device microbenchmarks):
- Head-constant RoPE folded into Wq/Wk on the host; 1/sqrt(D) into Wq.
- All matmuls fp32r (measured 158ns per [K<=128, M=128, N=512]); PE is the
  bottleneck engine at ~270us busy/core, ACT exp is second at ~216us.
- hT is shipped bf16 (halves the dominant 33MB input DMA; DMA measured
  ~225 GB/s/queue) and upconverted to f32r on DVE (measured ~0.27ns/elem).
- Scores are computed transposed ([ktok, qtok]); exp needs no max-
  subtraction; an all-ones 65th v-column makes the PV matmul emit the
  softmax denominator. E-matrix matmul broadcasts the reciprocal.
- Schedule: projection chunks run as bursts at unit boundaries (ACT has
  ~7us/unit of slack to drain its exp backlog there); PV lags scores by
  2 superblocks and crosses unit boundaries; o_proj of the previous unit
  is spread 2 PSUM-groups per superblock; qodd staging is prefetched one
  unit ahead on Pool. PSUM: 4 banks st (2x[128,1024]) + 4 banks attnT
  accumulators; proj/norm/o transients reuse those tags at points where
  the previous occupant is provably drained.
"""

import numpy as np
import ml_dtypes

import concourse.bass as bass
import concourse.bacc as bacc
import concourse.tile as tile
import concourse.mybir as mybir
from concourse import bass_utils

BF16 = mybir.dt.bfloat16
F32 = mybir.dt.float32
F32R = mybir.dt.float32r
bf16 = ml_dtypes.bfloat16

# Problem dims (hardcoded per contract)
B, S, HID = 2, 2048, 2048
NH, KVH, HD = 32, 8, 64
N_CORES = 8


class Dims:
    """All derived tile counts; parameterized so small variants can be
    simulated in CoreSim."""

    def __init__(self, B=B, S=S, HID=HID, n_qheads=4, HD=HD, out_ch=HID):
        self.B, self.S, self.HID, self.HD = B, S, HID, HD
        self.NQ = n_qheads              # q heads per core (must be 4)
        self.BS = B * S                 # total tokens
        self.QCH = n_qheads * HD        # q channels per core (256)
        self.OUT = out_ch               # o_proj output channels (2048)
        self.TOK_CHUNK = 512            # projection/attention token chunk
        self.KT = HID // 128            # contraction tiles for projections
        self.N_TC = self.BS // self.TOK_CHUNK
        self.N_QC = S // self.TOK_CHUNK  # q chunks per batch
        self.N_KB = S // 128            # ktok blocks per batch
        self.NSB = self.N_KB // 2       # superblocks (2 ktok blocks each)
        assert n_qheads == 4 and HD == 64
        assert self.OUT % 1024 == 0 and S % 512 == 0


def build_program(d: Dims, repeat: int = 1, et_bufs: int = 11):
    """Emit the per-core SPMD program. Returns compiled nc."""
    nc = bacc.Bacc("TRN2", target_bir_lowering=False, debug=False)

    # ---- DRAM I/O -------------------------------------------------------
    hT = nc.dram_tensor("hT", [d.HID, d.BS], BF16, kind="ExternalInput")
    wq = nc.dram_tensor("wq", [d.HID, d.QCH], BF16, kind="ExternalInput")
    wkv = nc.dram_tensor("wkv", [d.HID, 2 * d.HD], BF16, kind="ExternalInput")
    wo = nc.dram_tensor("wo", [d.QCH, d.OUT], BF16, kind="ExternalInput")
    emat = nc.dram_tensor("emat", [128, 256], F32R, kind="ExternalInput")
    id66 = nc.dram_tensor("id66", [66, 66], F32R, kind="ExternalInput")
    vones = nc.dram_tensor("vones", [2, d.BS], F32R, kind="ExternalInput")
    rzero = nc.dram_tensor("rzero", [128, 512], F32R, kind="ExternalInput")
    out = nc.dram_tensor("out", [d.BS, d.OUT], BF16, kind="ExternalOutput")

    with tile.TileContext(nc) as tc:
        with (
            tc.tile_pool(name="consts", bufs=1) as consts,
            tc.tile_pool(name="persist", bufs=1) as persist,
            tc.tile_pool(name="htb", bufs=19) as htb_pool,
            tc.tile_pool(name="htt", bufs=4) as htt_pool,
            tc.tile_pool(name="expp", bufs=et_bufs) as exp_pool,
            tc.tile_pool(name="norm", bufs=1) as norm_pool,
            tc.tile_pool(name="ostage", bufs=3) as ostage_pool,
            tc.tile_pool(name="vt", bufs=2) as vt_pool,
            tc.tile_pool(name="qstage", bufs=2) as qstage_pool,
            tc.tile_pool(name="wstage", bufs=3) as wstage_pool,
            tc.tile_pool(name="big", bufs=2, space="PSUM") as big_psum,
            tc.tile_pool(name="half", bufs=4, space="PSUM") as half_psum,
        ):
            # ---- weights: bf16 DMA staged via wstage, upconverted on DVE
            # (wo deferred into the pipeline) ------------------------------
            wq_sb = consts.tile([128, d.KT * d.QCH], F32R, tag="wq")
            wkv_sb = consts.tile([128, d.KT * 128], F32R, tag="wkv")
            emat_sb = consts.tile([128, 256], F32R, tag="emat")
            nc.scalar.dma_start(emat_sb[:], emat[:])
            id66_sb = consts.tile([66, 66], F32R, tag="id66")
            nc.scalar.dma_start(id66_sb[:], id66[:])
            wo_sb = consts.tile([128, 2 * d.OUT], F32R, tag="wo")

            def emit_wo_dma():
                # on the ACT hwdge queue: idle during the ramp, and 1MB here
                # would delay the hT chunk prefetches on the SP queue
                for ct in range(2):
                    for j in range(d.OUT // 512):
                        wob = wstage_pool.tile([128, 512], BF16, name="wob")
                        nc.scalar.dma_start(
                            wob[:], wo[ct * 128:(ct + 1) * 128,
                                       j * 512:(j + 1) * 512])
                        nc.vector.tensor_copy(
                            wo_sb[:, ct * d.OUT + j * 512:
                                  ct * d.OUT + (j + 1) * 512], wob[:])

            wq_stage = {}

            def emit_w_dma(kt):
                """One kt-slice of wq+wkv: bf16 stage (upconvert is JIT'd
                into burst 0 so it doesn't clog the DVE queue)."""
                wqb = wstage_pool.tile([128, 512], BF16, name="wqb")
                nc.sync.dma_start(wqb[:, 0:d.QCH], wq[kt * 128:(kt + 1) * 128, :])
                wkb = wstage_pool.tile([128, 512], BF16, name="wkb")
                nc.sync.dma_start(wkb[:, 0:128], wkv[kt * 128:(kt + 1) * 128, :])
                wq_stage[kt] = (wqb, wkb)

            def emit_w_up(kt):
                wqb, wkb = wq_stage.pop(kt)
                nc.vector.tensor_copy(
                    wq_sb[:, kt * d.QCH:(kt + 1) * d.QCH], wqb[:, 0:d.QCH])
                nc.vector.tensor_copy(
                    wkv_sb[:, kt * 128:(kt + 1) * 128], wkb[:, 0:128])

            # ---- persistent activations --------------------------------
            qT_sb = [persist.tile([128, d.BS], F32R, tag=f"qT{p}",
                                  name=f"qT{p}") for p in range(2)]
            kT_sb = persist.tile([64, d.BS], F32R, tag="kT")
            v_sb = persist.tile([128, (d.BS // 128) * (d.HD + 2)], F32R,
                                tag="v")
            attnT_sb = [persist.tile([128, d.TOK_CHUNK], F32R,
                                     tag=f"attnT{p}", name=f"attnT{p}")
                        for p in range(2)]
            recip_sb = persist.tile([128, d.TOK_CHUNK], F32R, tag="recip")
            nc.sync.dma_start(recip_sb[:], rzero[:])

            # ---- emission helpers --------------------------------------
            htb_tiles = {}   # (chunk, kt) -> bf16 tile
            htt_tiles = {}   # (chunk, kt) -> pre-upconverted f32r tile

            def emit_ht_dma(tcx, kts, split=False):
                cols = slice(tcx * d.TOK_CHUNK, (tcx + 1) * d.TOK_CHUNK)
                for kt in kts:
                    hb = htb_pool.tile([128, d.TOK_CHUNK], BF16, name="hb")
                    eng = nc.scalar if (split and kt % 2) else nc.sync
                    eng.dma_start(hb[:], hT[kt * 128:(kt + 1) * 128, cols])
                    htb_tiles[(tcx, kt)] = hb

            def emit_preup(tcx, kts):
                """Upconvert a few hT tiles ahead of a burst, so the burst's
                first matmuls don't wait on DVE work queued behind norm."""
                for kt in kts:
                    htt = htt_pool.tile([128, d.TOK_CHUNK], F32R, name="htt")
                    nc.vector.tensor_copy(htt[:], htb_tiles[(tcx, kt)][:])
                    htt_tiles[(tcx, kt)] = htt

            def emit_proj_burst(tcx, prefetch=None, presplit=False,
                                 mid=None):
                """Full projection of one 512-token chunk; hT bf16 tiles must
                be prefetched. prefetch: chunk whose hT loads are emitted
                just-in-time as this chunk's tiles free up. mid: callback
                emitted between the pq pass and the pkv pass (used to place
                the previous unit's norm_b so its DVE recip never blocks)."""
                cols = slice(tcx * d.TOK_CHUNK, (tcx + 1) * d.TOK_CHUNK)
                pq = big_psum.tile([128, 1024], F32, tag="big", name="pq")
                pkv = half_psum.tile([128, 512], F32, tag="half", name="pkv")
                lead = 4
                pkv_delay = []

                def flush_pkv(k2, h2, last=False):
                    nc.tensor.matmul(
                        pkv[:], wkv_sb[:, k2 * 128:(k2 + 1) * 128], h2[:],
                        start=(k2 == 0), stop=last)

                for kt in range(d.KT):
                    if prefetch is not None:
                        pk = kt + lead
                        if kt == 0:
                            emit_ht_dma(prefetch, range(min(lead, d.KT)),
                                        split=presplit)
                        if pk < d.KT:
                            emit_ht_dma(prefetch, [pk], split=presplit)
                    if kt in wq_stage:
                        emit_w_up(kt)
                    if (tcx, kt) in htt_tiles:
                        htt = htt_pool_tile = htt_tiles.pop((tcx, kt))
                    else:
                        htt = htt_pool.tile([128, d.TOK_CHUNK], F32R,
                                            name="htt")
                        nc.vector.tensor_copy(htt[:], htb_tiles[(tcx, kt)][:])
                    fl = dict(start=(kt == 0), stop=(kt == d.KT - 1))
                    nc.tensor.matmul(
                        pq[:, 0:512], wq_sb[:, kt * d.QCH: kt * d.QCH + 128],
                        htt[:], **fl)
                    nc.tensor.matmul(
                        pq[:, 512:1024],
                        wq_sb[:, kt * d.QCH + 128: kt * d.QCH + 256],
                        htt[:], **fl)
                    if mid is None:
                        # pkv mms trail by 2 kts (their PSUM slot can wait on
                        # the previous unit's norm muls at a boundary)
                        pkv_delay.append((kt, htt))
                        if len(pkv_delay) > 2:
                            flush_pkv(*pkv_delay.pop(0))
                        htb_tiles.pop((tcx, kt))
                if mid is None:
                    for i, (k2, h2) in enumerate(pkv_delay):
                        flush_pkv(k2, h2, last=(k2 == d.KT - 1))
                else:
                    mid()
                    # pkv pass: re-upconvert (htt tiles were recycled)
                    for kt in range(d.KT):
                        h2 = htt_pool.tile([128, d.TOK_CHUNK], F32R,
                                           name="htt")
                        nc.vector.tensor_copy(h2[:], htb_tiles[(tcx, kt)][:])
                        flush_pkv(kt, h2, last=(kt == d.KT - 1))
                        htb_tiles.pop((tcx, kt))
                nc.vector.tensor_copy(qT_sb[0][:, cols], pq[:, 0:512])
                nc.vector.tensor_copy(qT_sb[1][:, cols], pq[:, 512:1024])
                nc.vector.tensor_copy(kT_sb[:, cols], pkv[0:64, :])
                vt = vt_pool.tile([66, d.TOK_CHUNK], F32R, name="vt")
                nc.vector.tensor_copy(vt[0:64, :], pkv[64:128, :])
                nc.sync.dma_start(vt[64:66, :], vones[:, cols])
                for j in range(d.TOK_CHUNK // 128):
                    blk = tcx * (d.TOK_CHUNK // 128) + j
                    ptv = half_psum.tile([128, 512], F32R, tag="half",
                                         name="ptv")
                    nc.tensor.transpose(
                        ptv[0:128, 0:66],
                        vt[0:66, j * 128:(j + 1) * 128], id66_sb[:])
                    nc.vector.tensor_copy(
                        v_sb[:, blk * 66:(blk + 1) * 66], ptv[0:128, 0:66])

            def emit_qodd(u):
                u["qodd"] = [qstage_pool.tile([64, d.TOK_CHUNK], F32R,
                                              name=f"qodd{p}")
                             for p in range(2)]
                for p in range(2):
                    nc.gpsimd.tensor_copy(
                        u["qodd"][p][:, :], qT_sb[p][64:128, u["qcols"]])

            def emit_scores_kb(u, kb, expT):
                b = u["b"]
                kcols = slice(b * d.S + kb * 128, b * d.S + (kb + 1) * 128)
                for p in range(2):
                    st = big_psum.tile([128, 1024], F32, tag="big",
                                       name="st")
                    for hh in range(2):
                        rhs = (qT_sb[p][0:64, u["qcols"]] if hh == 0
                               else u["qodd"][p][:, :])
                        nc.tensor.matmul(
                            st[:, 512 * hh:512 * hh + 512],
                            kT_sb[:, kcols], rhs,
                            start=True, stop=True)
                    et = exp_pool.tile([128, 1024], F32R, name="et")
                    nc.scalar.activation(
                        et[:], st[:], mybir.ActivationFunctionType.Exp)
                    expT[kb, p] = et

            def emit_pv_kb(u, kb, expT):
                b = u["b"]
                vblk = (b * d.S) // 128 + kb
                for h in range(4):
                    nc.tensor.matmul(
                        u["attnT_ps"][h][:],
                        v_sb[:, vblk * 66:vblk * 66 + 65],
                        expT[kb, h // 2][:, 512 * (h % 2):
                                         512 * (h % 2) + 512],
                        start=(kb == 0), stop=(kb == d.N_KB - 1))

            def emit_pv_sb(u, sb, expT):
                for kb in range(2 * sb, 2 * sb + 2):
                    emit_pv_kb(u, kb, expT)

            def emit_norm_a(u):
                """Denominator collection + E-matrix broadcast matmuls."""
                attnT_ps = u["attnT_ps"]
                for h in range(4):
                    nc.vector.tensor_copy(
                        recip_sb[32 * h:32 * h + 1, :], attnT_ps[h][64:65, :])
                u["bcast_ps"] = [big_psum.tile([128, 512], F32, tag="big",
                                               name=f"bcast_ps{p}")
                                 for p in range(2)]
                for p in range(2):
                    nc.tensor.matmul(
                        u["bcast_ps"][p][:], emat_sb[:, 128 * p:128 * (p + 1)],
                        recip_sb[:], start=True, stop=True)

            def emit_norm_b(u):
                """Reciprocal + normalization muls. Emitted with PE work in
                between so the custom-DVE recip never head-of-line blocks the
                DVE queue waiting on the broadcast matmul."""
                attnT_ps = u["attnT_ps"]
                bcast_sb = [norm_pool.tile([128, 512], F32,
                                           name=f"bcast_sb{p}")
                            for p in range(2)]
                for p in range(2):
                    nc.vector.reciprocal_approx_fast(
                        out=bcast_sb[p][:], in_=u["bcast_ps"][p][:])
                for p in range(2):
                    for hh in range(2):
                        nc.vector.tensor_mul(
                            attnT_sb[p][64 * hh:64 * hh + 64, :],
                            attnT_ps[2 * p + hh][0:64, :],
                            bcast_sb[p][64 * hh:64 * hh + 64, :])

            def emit_o_group(u, g):
                """One PSUM group of the o_proj: 128 tokens x 1024 out-ch."""
                b, qc = u["b"], u["qc"]
                qs, nh = g // (d.OUT // 1024), g % (d.OUT // 1024)
                rows = slice(b * d.S + qc * d.TOK_CHUNK + qs * 128,
                             b * d.S + qc * d.TOK_CHUNK + (qs + 1) * 128)
                po = big_psum.tile([128, 1024], F32, tag="big", name="po")
                for ct in range(2):
                    for nn in range(2):
                        nc.tensor.matmul(
                            po[:, nn * 512:(nn + 1) * 512],
                            attnT_sb[ct][:, qs * 128:(qs + 1) * 128],
                            wo_sb[:, ct * d.OUT + nh * 1024 + nn * 512:
                                  ct * d.OUT + nh * 1024 + (nn + 1) * 512],
                            start=(ct == 0), stop=(ct == 1))
                ot = ostage_pool.tile([128, 1024], BF16, name="ot")
                nc.vector.tensor_copy(ot[:], po[:])
                nc.sync.dma_start(
                    out[rows, nh * 1024:(nh + 1) * 1024], ot[:])

            NG = (d.TOK_CHUNK // 128) * (d.OUT // 1024)  # o groups per unit

            for _rep in range(repeat):
                units = []
                for b_ in range(d.B):
                    for qc in range(d.N_QC):
                        units.append({
                            "b": b_, "qc": qc,
                            "qcols": slice(b_ * d.S + qc * d.TOK_CHUNK,
                                           b_ * d.S + (qc + 1) * d.TOK_CHUNK),
                        })
                n_units = len(units)
                # upfront: batch-0 projection chunks; hT split across the SP
                # and ACT hwdge queues so the ramp is not DMA-bound
                upfront = d.N_QC if d.B > 1 else d.N_TC
                for kt in range(d.KT):
                    emit_w_dma(kt)
                    emit_ht_dma(0, [kt], split=True)
                for c in range(upfront):
                    # full-chunk-ahead prefetch on both hwdge queues (no exp
                    # traffic on the ACT queue yet); htb slot waits pace the
                    # queues to this burst's consumption automatically
                    if c + 1 <= upfront and c + 1 < d.N_TC:
                        emit_ht_dma(c + 1, range(d.KT), split=True)
                    emit_proj_burst(c)
                    if c == 0:
                        emit_wo_dma()
                        if d.B > 1:
                            emit_qodd(units[0])

                if d.B == 1:
                    emit_qodd(units[0])
                prev = None
                norm_b_pending = None
                for ui, u in enumerate(units):
                    bc = upfront + 1 + ui
                    do_burst = d.B > 1 and upfront + ui < d.N_TC
                    if do_burst:
                        emit_preup(upfront + ui, range(4))
                    pv = prev
                    if pv is not None:
                        emit_pv_sb(pv, d.NSB - 2, pv["expT"])
                        emit_pv_sb(pv, d.NSB - 1, pv["expT"])
                        emit_norm_a(pv)
                    # boundary projection burst (batch-1 chunks)
                    if do_burst:
                        emit_proj_burst(
                            upfront + ui,
                            mid=(lambda pv=pv: emit_norm_b(pv))
                            if pv is not None else None)
                    elif pv is not None:
                        norm_b_pending = pv
                    u["attnT_ps"] = [
                        half_psum.tile([65, 512], F32, tag="half",
                                       name=f"attnT_ps{h}") for h in range(4)]
                    u["expT"] = {}
                    po_slots = {2 + (j * (2 * d.NSB - 3)) // NG: j
                                for j in range(NG)}
                    for sb in range(d.NSB):
                        for i, kb in enumerate((2 * sb, 2 * sb + 1)):
                            emit_scores_kb(u, kb, u["expT"])
                            if norm_b_pending is not None:
                                emit_norm_b(norm_b_pending)
                                norm_b_pending = None
                            if sb >= 2:
                                emit_pv_kb(u, 2 * (sb - 2) + i, u["expT"])
                            # previous unit's o_proj, spread over the unit
                            if prev is not None:
                                g = po_slots.get(2 * sb + i)
                                if g is not None:
                                    emit_o_group(prev, g)
                        # prefetch next boundary chunk's hT, 2 kts per sb
                        if d.B > 1 and bc < d.N_TC and 2 * sb < d.KT:
                            emit_ht_dma(bc, range(
                                2 * sb, min(2 * sb + 2, d.KT)))
                        if sb == d.NSB - 3 and ui + 1 < n_units:
                            emit_qodd(units[ui + 1])
                    prev = u
                emit_pv_sb(prev, d.NSB - 2, prev["expT"])
                emit_pv_sb(prev, d.NSB - 1, prev["expT"])
                emit_norm_a(prev)
                emit_norm_b(prev)
                for g in range(NG):
                    emit_o_group(prev, g)

    nc.compile()
    return nc


def _rope_fold(W, cos, sin, nheads, scale):
    """Fold head-constant RoPE (and scale) into a projection weight.
    W: [HID, nheads*64] fp32; cos/sin: [nheads, 64]."""
    W4 = W.reshape(W.shape[0], nheads, 64)
    out = np.empty_like(W4)
    out[:, :, :32] = W4[:, :, :32] * cos[None, :, :32] \
        - W4[:, :, 32:] * sin[None, :, :32]
    out[:, :, 32:] = W4[:, :, 32:] * cos[None, :, 32:] \
        + W4[:, :, :32] * sin[None, :, 32:]
    return (out * scale).reshape(W.shape)


_PROGRAM_CACHE = {}


def _get_program():
    if "nc" not in _PROGRAM_CACHE:
        _PROGRAM_CACHE["nc"] = build_program(Dims())
    return _PROGRAM_CACHE["nc"]


def make_in_maps(hidden_states, Wq, Wk, Wv, Wo, cos, sin, d: Dims = None):
    """Host-side sharding/prep. Returns per-core input dicts."""
    d = d or Dims()
    hs = np.asarray(hidden_states, np.float32).reshape(d.BS, d.HID)
    hT = np.ascontiguousarray(hs.T).astype(bf16)
    cos = np.asarray(cos, np.float32)
    sin = np.asarray(sin, np.float32)
    nq_total = N_CORES * d.NQ
    Wq_f = _rope_fold(np.asarray(Wq, np.float32), cos[:nq_total],
                      sin[:nq_total], nq_total, 1.0 / np.sqrt(d.HD))
    Wk_f = _rope_fold(np.asarray(Wk, np.float32), cos[:KVH], sin[:KVH],
                      KVH, 1.0)
    Wv_f = np.asarray(Wv, np.float32)
    Wo_f = np.asarray(Wo, np.float32)
    emat = np.zeros([128, 256], np.float32)
    for h in range(4):
        p, hh = h // 2, h % 2
        emat[32 * h, 128 * p + 64 * hh:128 * p + 64 * hh + 64] = 1.0
    id66 = np.eye(66, dtype=np.float32)
    vones = np.concatenate([np.ones([1, d.BS], np.float32),
                            np.zeros([1, d.BS], np.float32)])
    rzero = np.zeros([128, 512], np.float32)
    in_maps = []
    for c in range(N_CORES):
        wq_c = np.ascontiguousarray(
            Wq_f[:, c * d.QCH:(c + 1) * d.QCH]).astype(bf16)
        wkv_c = np.ascontiguousarray(np.concatenate(
            [Wk_f[:, c * d.HD:(c + 1) * d.HD],
             Wv_f[:, c * d.HD:(c + 1) * d.HD]], axis=1)).astype(bf16)
        wo_c = np.ascontiguousarray(
            Wo_f[c * d.QCH:(c + 1) * d.QCH, :]).astype(bf16)
        in_maps.append({
            "hT": hT, "wq": wq_c, "wkv": wkv_c, "wo": wo_c,
            "emat": emat, "id66": id66, "vones": vones,
            "rzero": rzero,
        })
    return in_maps


def kernel(hidden_states, Wq, Wk, Wv, Wo, cos, sin):
    d = Dims()
    nc = _get_program()
    in_maps = make_in_maps(hidden_states, Wq, Wk, Wv, Wo, cos, sin, d)
    res = bass_utils.run_bass_kernel_spmd(
        nc, in_maps, core_ids=list(range(N_CORES)))
    acc = res.results[0]["out"].astype(np.float32)
    for c in range(1, N_CORES):
        acc += res.results[c]["out"].astype(np.float32)
    return acc.reshape(B, S, HID)


if __name__ == "__main__":
    rng = np.random.default_rng(0)
    h = rng.standard_normal((B, S, HID), dtype=np.float32)
    sc = 1.0 / np.sqrt(HID)
    Wq_ = rng.standard_normal((HID, NH * HD), dtype=np.float32) * sc
    Wk_ = rng.standard_normal((HID, KVH * HD), dtype=np.float32) * sc
    Wv_ = rng.standard_normal((HID, KVH * HD), dtype=np.float32) * sc
    Wo_ = rng.standard_normal((NH * HD, HID), dtype=np.float32) * sc
    inv = 1.0 / (10000.0 ** (np.arange(0, HD, 2, dtype=np.float32) / HD))
    t = np.arange(S, dtype=np.float32)
    fr = np.outer(t, inv)
    emb = np.concatenate([fr, fr], axis=-1)
    o = kernel(h, Wq_, Wk_, Wv_, Wo_, np.cos(emb), np.sin(emb))
    print("out", o.shape, o.dtype, float(np.abs(o).max()))


# revision 21
# speedup vs baseline: 2.3247x; 2.3247x over previous
"""Trainium2 Bass kernel for nn_AvaAttention (GQA attention, head-constant RoPE).

Sharding: tensor-parallel over the 8 kv heads -> core c owns kv head c and
q heads 4c..4c+3. Each core computes its 4 heads' attention and a partial
o_proj (row-split Wo); the host sums the 8 partials.

Key device-side design points (v2, rates from on-device microbenchmarks):
- Head-constant RoPE folded into Wq/Wk on the host; 1/sqrt(D) into Wq.
- All matmuls fp32r (measured 158ns per [K<=128, M=128, N=512]); PE is the
  bottleneck engine at ~270us busy/core, ACT exp is second at ~216us.
- hT is shipped bf16 (halves the dominant 33MB input DMA; DMA measured
  ~225 GB/s/queue) and upconverted to f32r on DVE (measured ~0.27ns/elem).
- Scores are computed transposed ([ktok, qtok]); exp needs no max-
  subtraction; an all-ones 65th v-column makes the PV matmul emit the
  softmax denominator. E-matrix matmul broadcasts the reciprocal.
- Schedule: projection chunks run as bursts at unit boundaries (ACT has
  ~7us/unit of slack to drain its exp backlog there); PV lags scores by
  2 superblocks and crosses unit boundaries; o_proj of the previous unit
  is spread 2 PSUM-groups per superblock; qodd staging is prefetched one
  unit ahead on Pool. PSUM: 4 banks st (2x[128,1024]) + 4 banks attnT
  accumulators; proj/norm/o transients reuse those tags at points where
  the previous occupant is provably drained.
"""

import numpy as np
import ml_dtypes

import concourse.bass as bass
import concourse.bacc as bacc
import concourse.tile as tile
import concourse.mybir as mybir
from concourse import bass_utils

BF16 = mybir.dt.bfloat16
F32 = mybir.dt.float32
F32R = mybir.dt.float32r
bf16 = ml_dtypes.bfloat16

# Problem dims (hardcoded per contract)
B, S, HID = 2, 2048, 2048
NH, KVH, HD = 32, 8, 64
N_CORES = 8


class Dims:
    """All derived tile counts; parameterized so small variants can be
    simulated in CoreSim."""

    def __init__(self, B=B, S=S, HID=HID, n_qheads=4, HD=HD, out_ch=HID):
        self.B, self.S, self.HID, self.HD = B, S, HID, HD
        self.NQ = n_qheads              # q heads per core (must be 4)
        self.BS = B * S                 # total tokens
        self.QCH = n_qheads * HD        # q channels per core (256)
        self.OUT = out_ch               # o_proj output channels (2048)
        self.TOK_CHUNK = 512            # projection/attention token chunk
        self.KT = HID // 128            # contraction tiles for projections
        self.N_TC = self.BS // self.TOK_CHUNK
        self.N_QC = S // self.TOK_CHUNK  # q chunks per batch
        self.N_KB = S // 128            # ktok blocks per batch
        self.NSB = self.N_KB // 2       # superblocks (2 ktok blocks each)
        assert n_qheads == 4 and HD == 64
        assert self.OUT % 1024 == 0 and S % 512 == 0


def build_program(d: Dims, repeat: int = 1, et_bufs: int = 11):
    """Emit the per-core SPMD program. Returns compiled nc."""
    nc = bacc.Bacc("TRN2", target_bir_lowering=False, debug=False)

    # ---- DRAM I/O -------------------------------------------------------
    hT = nc.dram_tensor("hT", [d.HID, d.BS], BF16, kind="ExternalInput")
    wq = nc.dram_tensor("wq", [d.HID, d.QCH], BF16, kind="ExternalInput")
    wkv = nc.dram_tensor("wkv", [d.HID, 2 * d.HD], BF16, kind="ExternalInput")
    wo = nc.dram_tensor("wo", [d.QCH, d.OUT], BF16, kind="ExternalInput")
    emat = nc.dram_tensor("emat", [128, 256], F32R, kind="ExternalInput")
    id66 = nc.dram_tensor("id66", [66, 66], F32R, kind="ExternalInput")
    vones = nc.dram_tensor("vones", [2, d.BS], F32R, kind="ExternalInput")
    rzero = nc.dram_tensor("rzero", [128, 512], F32R, kind="ExternalInput")
    out = nc.dram_tensor("out", [d.BS, d.OUT], BF16, kind="ExternalOutput")

    with tile.TileContext(nc) as tc:
        with (
            tc.tile_pool(name="consts", bufs=1) as consts,
            tc.tile_pool(name="persist", bufs=1) as persist,
            tc.tile_pool(name="htb", bufs=19) as htb_pool,
            tc.tile_pool(name="htt", bufs=4) as htt_pool,
            tc.tile_pool(name="expp", bufs=et_bufs) as exp_pool,
            tc.tile_pool(name="norm", bufs=1) as norm_pool,
            tc.tile_pool(name="ostage", bufs=3) as ostage_pool,
            tc.tile_pool(name="vt", bufs=2) as vt_pool,
            tc.tile_pool(name="qstage", bufs=2) as qstage_pool,
            tc.tile_pool(name="wstage", bufs=3) as wstage_pool,
            tc.tile_pool(name="big", bufs=2, space="PSUM") as big_psum,
            tc.tile_pool(name="half", bufs=4, space="PSUM") as half_psum,
        ):
            # ---- weights: bf16 DMA staged via wstage, upconverted on DVE
            # (wo deferred into the pipeline) ------------------------------
            wq_sb = consts.tile([128, d.KT * d.QCH], F32R, tag="wq")
            wkv_sb = consts.tile([128, d.KT * 128], F32R, tag="wkv")
            emat_sb = consts.tile([128, 256], F32R, tag="emat")
            nc.scalar.dma_start(emat_sb[:], emat[:])
            id66_sb = consts.tile([66, 66], F32R, tag="id66")
            nc.scalar.dma_start(id66_sb[:], id66[:])
            wo_sb = consts.tile([128, 2 * d.OUT], F32R, tag="wo")

            def emit_wo_dma():
                # on the ACT hwdge queue: idle during the ramp, and 1MB here
                # would delay the hT chunk prefetches on the SP queue
                for ct in range(2):
                    for j in range(d.OUT // 512):
                        wob = wstage_pool.tile([128, 512], BF16, name="wob")
                        nc.scalar.dma_start(
                            wob[:], wo[ct * 128:(ct + 1) * 128,
                                       j * 512:(j + 1) * 512])
                        nc.vector.tensor_copy(
                            wo_sb[:, ct * d.OUT + j * 512:
                                  ct * d.OUT + (j + 1) * 512], wob[:])

            wq_stage = {}

            def emit_w_dma(kt):
                """One kt-slice of wq+wkv: bf16 stage (upconvert is JIT'd
                into burst 0 so it doesn't clog the DVE queue)."""
                wqb = wstage_pool.tile([128, 512], BF16, name="wqb")
                nc.sync.dma_start(wqb[:, 0:d.QCH], wq[kt * 128:(kt + 1) * 128, :])
                wkb = wstage_pool.tile([128, 512], BF16, name="wkb")
                nc.sync.dma_start(wkb[:, 0:128], wkv[kt * 128:(kt + 1) * 128, :])
                wq_stage[kt] = (wqb, wkb)

            def emit_w_up(kt):
                wqb, wkb = wq_stage.pop(kt)
                nc.vector.tensor_copy(
                    wq_sb[:, kt * d.QCH:(kt + 1) * d.QCH], wqb[:, 0:d.QCH])
                nc.vector.tensor_copy(
                    wkv_sb[:, kt * 128:(kt + 1) * 128], wkb[:, 0:128])

            # ---- persistent activations --------------------------------
            qT_sb = [persist.tile([128, d.BS], F32R, tag=f"qT{p}",
                                  name=f"qT{p}") for p in range(2)]
            kT_sb = persist.tile([64, d.BS], F32R, tag="kT")
            v_sb = persist.tile([128, (d.BS // 128) * (d.HD + 2)], F32R,
                                tag="v")
            attnT_sb = [persist.tile([128, d.TOK_CHUNK], F32R,
                                     tag=f"attnT{p}", name=f"attnT{p}")
                        for p in range(2)]
            recip_sb = persist.tile([128, d.TOK_CHUNK], F32R, tag="recip")
            nc.sync.dma_start(recip_sb[:], rzero[:])

            # ---- emission helpers --------------------------------------
            htb_tiles = {}   # (chunk, kt) -> bf16 tile
            htt_tiles = {}   # (chunk, kt) -> pre-upconverted f32r tile

            def emit_ht_dma(tcx, kts, split=False):
                cols = slice(tcx * d.TOK_CHUNK, (tcx + 1) * d.TOK_CHUNK)
                for kt in kts:
                    hb = htb_pool.tile([128, d.TOK_CHUNK], BF16, name="hb")
                    eng = nc.scalar if (split and kt % 2) else nc.sync
                    eng.dma_start(hb[:], hT[kt * 128:(kt + 1) * 128, cols])
                    htb_tiles[(tcx, kt)] = hb

            def emit_preup(tcx, kts):
                """Upconvert a few hT tiles ahead of a burst, so the burst's
                first matmuls don't wait on DVE work queued behind norm."""
                for kt in kts:
                    htt = htt_pool.tile([128, d.TOK_CHUNK], F32R, name="htt")
                    nc.vector.tensor_copy(htt[:], htb_tiles[(tcx, kt)][:])
                    htt_tiles[(tcx, kt)] = htt

            def emit_proj_burst(tcx, prefetch=None, presplit=False,
                                 mid=None):
                """Full projection of one 512-token chunk; hT bf16 tiles must
                be prefetched. prefetch: chunk whose hT loads are emitted
                just-in-time as this chunk's tiles free up. mid: callback
                emitted between the pq pass and the pkv pass (used to place
                the previous unit's norm_b so its DVE recip never blocks)."""
                cols = slice(tcx * d.TOK_CHUNK, (tcx + 1) * d.TOK_CHUNK)
                pq = big_psum.tile([128, 1024], F32, tag="big", name="pq")
                pkv = half_psum.tile([128, 512], F32, tag="half", name="pkv")
                lead = 4
                pkv_delay = []

                def flush_pkv(k2, h2, last=False):
                    nc.tensor.matmul(
                        pkv[:], wkv_sb[:, k2 * 128:(k2 + 1) * 128], h2[:],
                        start=(k2 == 0), stop=last)

                for kt in range(d.KT):
                    if prefetch is not None:
                        pk = kt + lead
                        if kt == 0:
                            emit_ht_dma(prefetch, range(min(lead, d.KT)),
                                        split=presplit)
                        if pk < d.KT:
                            emit_ht_dma(prefetch, [pk], split=presplit)
                    if kt in wq_stage:
                        emit_w_up(kt)
                    if (tcx, kt) in htt_tiles:
                        htt = htt_pool_tile = htt_tiles.pop((tcx, kt))
                    else:
                        htt = htt_pool.tile([128, d.TOK_CHUNK], F32R,
                                            name="htt")
                        nc.vector.tensor_copy(htt[:], htb_tiles[(tcx, kt)][:])
                    fl = dict(start=(kt == 0), stop=(kt == d.KT - 1))
                    nc.tensor.matmul(
                        pq[:, 0:512], wq_sb[:, kt * d.QCH: kt * d.QCH + 128],
                        htt[:], **fl)
                    nc.tensor.matmul(
                        pq[:, 512:1024],
                        wq_sb[:, kt * d.QCH + 128: kt * d.QCH + 256],
                        htt[:], **fl)
                    if mid is None:
                        # pkv mms trail by 2 kts (their PSUM slot can wait on
                        # the previous unit's norm muls at a boundary)
                        pkv_delay.append((kt, htt))
                        if len(pkv_delay) > 2:
                            flush_pkv(*pkv_delay.pop(0))
                        htb_tiles.pop((tcx, kt))
                if mid is None:
                    for i, (k2, h2) in enumerate(pkv_delay):
                        flush_pkv(k2, h2, last=(k2 == d.KT - 1))
                else:
                    mid()
                    # pkv pass: re-upconvert (htt tiles were recycled)
                    for kt in range(d.KT):
                        h2 = htt_pool.tile([128, d.TOK_CHUNK], F32R,
                                           name="htt")
                        nc.vector.tensor_copy(h2[:], htb_tiles[(tcx, kt)][:])
                        flush_pkv(kt, h2, last=(kt == d.KT - 1))
                        htb_tiles.pop((tcx, kt))
                nc.vector.tensor_copy(qT_sb[0][:, cols], pq[:, 0:512])
                nc.vector.tensor_copy(qT_sb[1][:, cols], pq[:, 512:1024])
                nc.vector.tensor_copy(kT_sb[:, cols], pkv[0:64, :])
                vt = vt_pool.tile([66, d.TOK_CHUNK], F32R, name="vt")
                nc.vector.tensor_copy(vt[0:64, :], pkv[64:128, :])
                nc.sync.dma_start(vt[64:66, :], vones[:, cols])
                for j in range(d.TOK_CHUNK // 128):
                    blk = tcx * (d.TOK_CHUNK // 128) + j
                    ptv = half_psum.tile([128, 512], F32R, tag="half",
                                         name="ptv")
                    nc.tensor.transpose(
                        ptv[0:128, 0:66],
                        vt[0:66, j * 128:(j + 1) * 128], id66_sb[:])
                    nc.vector.tensor_copy(
                        v_sb[:, blk * 66:(blk + 1) * 66], ptv[0:128, 0:66])

            def emit_qodd(u):
                u["qodd"] = [qstage_pool.tile([64, d.TOK_CHUNK], F32R,
                                              name=f"qodd{p}")
                             for p in range(2)]
                for p in range(2):
                    nc.gpsimd.tensor_copy(
                        u["qodd"][p][:, :], qT_sb[p][64:128, u["qcols"]])

            def emit_scores_kb(u, kb, expT):
                b = u["b"]
                kcols = slice(b * d.S + kb * 128, b * d.S + (kb + 1) * 128)
                for p in range(2):
                    st = big_psum.tile([128, 1024], F32, tag="big",
                                       name="st")
                    for hh in range(2):
                        rhs = (qT_sb[p][0:64, u["qcols"]] if hh == 0
                               else u["qodd"][p][:, :])
                        nc.tensor.matmul(
                            st[:, 512 * hh:512 * hh + 512],
                            kT_sb[:, kcols], rhs,
                            start=True, stop=True)
                    et = exp_pool.tile([128, 1024], F32R, name="et")
                    nc.scalar.activation(
                        et[:], st[:], mybir.ActivationFunctionType.Exp)
                    expT[kb, p] = et

            def emit_pv_kb(u, kb, expT):
                b = u["b"]
                vblk = (b * d.S) // 128 + kb
                for h in range(4):
                    nc.tensor.matmul(
                        u["attnT_ps"][h][:],
                        v_sb[:, vblk * 66:vblk * 66 + 65],
                        expT[kb, h // 2][:, 512 * (h % 2):
                                         512 * (h % 2) + 512],
                        start=(kb == 0), stop=(kb == d.N_KB - 1))

            def emit_pv_sb(u, sb, expT):
                for kb in range(2 * sb, 2 * sb + 2):
                    emit_pv_kb(u, kb, expT)

            def emit_norm_a(u):
                """Denominator collection + E-matrix broadcast matmuls."""
                attnT_ps = u["attnT_ps"]
                for h in range(4):
                    nc.vector.tensor_copy(
                        recip_sb[32 * h:32 * h + 1, :], attnT_ps[h][64:65, :])
                u["bcast_ps"] = [big_psum.tile([128, 512], F32, tag="big",
                                               name=f"bcast_ps{p}")
                                 for p in range(2)]
                for p in range(2):
                    nc.tensor.matmul(
                        u["bcast_ps"][p][:], emat_sb[:, 128 * p:128 * (p + 1)],
                        recip_sb[:], start=True, stop=True)

            def emit_norm_b(u):
                """Reciprocal + normalization muls. Emitted with PE work in
                between so the custom-DVE recip never head-of-line blocks the
                DVE queue waiting on the broadcast matmul."""
                attnT_ps = u["attnT_ps"]
                bcast_sb = [norm_pool.tile([128, 512], F32,
                                           name=f"bcast_sb{p}")
                            for p in range(2)]
                for p in range(2):
                    nc.vector.reciprocal_approx_fast(
                        out=bcast_sb[p][:], in_=u["bcast_ps"][p][:])
                for p in range(2):
                    for hh in range(2):
                        nc.vector.tensor_mul(
                            attnT_sb[p][64 * hh:64 * hh + 64, :],
                            attnT_ps[2 * p + hh][0:64, :],
                            bcast_sb[p][64 * hh:64 * hh + 64, :])

            def emit_o_group(u, g):
                """One PSUM group of the o_proj: 128 tokens x 1024 out-ch."""
                b, qc = u["b"], u["qc"]
                qs, nh = g // (d.OUT // 1024), g % (d.OUT // 1024)
                rows = slice(b * d.S + qc * d.TOK_CHUNK + qs * 128,
                             b * d.S + qc * d.TOK_CHUNK + (qs + 1) * 128)
                po = big_psum.tile([128, 1024], F32, tag="big", name="po")
                for ct in range(2):
                    for nn in range(2):
                        nc.tensor.matmul(
                            po[:, nn * 512:(nn + 1) * 512],
                            attnT_sb[ct][:, qs * 128:(qs + 1) * 128],
                            wo_sb[:, ct * d.OUT + nh * 1024 + nn * 512:
                                  ct * d.OUT + nh * 1024 + (nn + 1) * 512],
                            start=(ct == 0), stop=(ct == 1))
                ot = ostage_pool.tile([128, 1024], BF16, name="ot")
                nc.vector.tensor_copy(ot[:], po[:])
                nc.sync.dma_start(
                    out[rows, nh * 1024:(nh + 1) * 1024], ot[:])

            NG = (d.TOK_CHUNK // 128) * (d.OUT // 1024)  # o groups per unit

            for _rep in range(repeat):
                units = []
                for b_ in range(d.B):
                    for qc in range(d.N_QC):
                        units.append({
                            "b": b_, "qc": qc,
                            "qcols": slice(b_ * d.S + qc * d.TOK_CHUNK,
                                           b_ * d.S + (qc + 1) * d.TOK_CHUNK),
                        })
                n_units = len(units)
                # upfront: batch-0 projection chunks; hT split across the SP
                # and ACT hwdge queues so the ramp is not DMA-bound
                upfront = d.N_QC if d.B > 1 else d.N_TC
                for kt in range(d.KT):
                    emit_w_dma(kt)
                    emit_ht_dma(0, [kt], split=True)
                for c in range(upfront):
                    # full-chunk-ahead prefetch on both hwdge queues (no exp
                    # traffic on the ACT queue yet); htb slot waits pace the
                    # queues to this burst's consumption automatically
                    if c + 1 <= upfront and c + 1 < d.N_TC:
                        emit_ht_dma(c + 1, range(d.KT), split=True)
                    emit_proj_burst(c)
                    if c == 0:
                        emit_wo_dma()
                        if d.B > 1:
                            emit_qodd(units[0])

                if d.B == 1:
                    emit_qodd(units[0])
                prev = None
                norm_b_pending = None
                for ui, u in enumerate(units):
                    bc = upfront + 1 + ui
                    do_burst = d.B > 1 and upfront + ui < d.N_TC
                    if do_burst:
                        emit_preup(upfront + ui, range(4))
                    pv = prev
                    if pv is not None and not do_burst:
                        emit_pv_sb(pv, d.NSB - 2, pv["expT"])
                        emit_pv_sb(pv, d.NSB - 1, pv["expT"])
                        emit_norm_a(pv)
                        norm_b_pending = pv
                    if do_burst:
                        if pv is not None:
                            # last superblock of PV + norm go between the
                            # burst passes: the pq pass gives ACT time to
                            # drain its exp backlog before pv(sb7) needs it
                            emit_pv_sb(pv, d.NSB - 2, pv["expT"])

                            def _mid(pv=pv):
                                emit_pv_sb(pv, d.NSB - 1, pv["expT"])
                                emit_norm_a(pv)
                                emit_norm_b(pv)
                        else:
                            _mid = None
                        emit_proj_burst(upfront + ui, mid=_mid)
                    u["attnT_ps"] = [
                        half_psum.tile([65, 512], F32, tag="half",
                                       name=f"attnT_ps{h}") for h in range(4)]
                    u["expT"] = {}
                    po_slots = {2 + (j * (2 * d.NSB - 3)) // NG: j
                                for j in range(NG)}
                    for sb in range(d.NSB):
                        for i, kb in enumerate((2 * sb, 2 * sb + 1)):
                            emit_scores_kb(u, kb, u["expT"])
                            if norm_b_pending is not None:
                                emit_norm_b(norm_b_pending)
                                norm_b_pending = None
                            if sb >= 2:
                                emit_pv_kb(u, 2 * (sb - 2) + i, u["expT"])
                            # previous unit's o_proj, spread over the unit
                            if prev is not None:
                                g = po_slots.get(2 * sb + i)
                                if g is not None:
                                    emit_o_group(prev, g)
                        # prefetch next boundary chunk's hT, 2 kts per sb
                        if d.B > 1 and bc < d.N_TC and 2 * sb < d.KT:
                            emit_ht_dma(bc, range(
                                2 * sb, min(2 * sb + 2, d.KT)))
                        if sb == d.NSB - 3 and ui + 1 < n_units:
                            emit_qodd(units[ui + 1])
                    prev = u
                emit_pv_sb(prev, d.NSB - 2, prev["expT"])
                emit_pv_sb(prev, d.NSB - 1, prev["expT"])
                emit_norm_a(prev)
                emit_norm_b(prev)
                for g in range(NG):
                    emit_o_group(prev, g)

    nc.compile()
    return nc


def _rope_fold(W, cos, sin, nheads, scale):
    """Fold head-constant RoPE (and scale) into a projection weight.
    W: [HID, nheads*64] fp32; cos/sin: [nheads, 64]."""
    W4 = W.reshape(W.shape[0], nheads, 64)
    out = np.empty_like(W4)
    out[:, :, :32] = W4[:, :, :32] * cos[None, :, :32] \
        - W4[:, :, 32:] * sin[None, :, :32]
    out[:, :, 32:] = W4[:, :, 32:] * cos[None, :, 32:] \
        + W4[:, :, :32] * sin[None, :, 32:]
    return (out * scale).reshape(W.shape)


_PROGRAM_CACHE = {}


def _get_program():
    if "nc" not in _PROGRAM_CACHE:
        _PROGRAM_CACHE["nc"] = build_program(Dims())
    return _PROGRAM_CACHE["nc"]


def make_in_maps(hidden_states, Wq, Wk, Wv, Wo, cos, sin, d: Dims = None):
    """Host-side sharding/prep. Returns per-core input dicts."""
    d = d or Dims()
    hs = np.asarray(hidden_states, np.float32).reshape(d.BS, d.HID)
    hT = np.ascontiguousarray(hs.T).astype(bf16)
    cos = np.asarray(cos, np.float32)
    sin = np.asarray(sin, np.float32)
    nq_total = N_CORES * d.NQ
    Wq_f = _rope_fold(np.asarray(Wq, np.float32), cos[:nq_total],
                      sin[:nq_total], nq_total, 1.0 / np.sqrt(d.HD))
    Wk_f = _rope_fold(np.asarray(Wk, np.float32), cos[:KVH], sin[:KVH],
                      KVH, 1.0)
    Wv_f = np.asarray(Wv, np.float32)
    Wo_f = np.asarray(Wo, np.float32)
    emat = np.zeros([128, 256], np.float32)
    for h in range(4):
        p, hh = h // 2, h % 2
        emat[32 * h, 128 * p + 64 * hh:128 * p + 64 * hh + 64] = 1.0
    id66 = np.eye(66, dtype=np.float32)
    vones = np.concatenate([np.ones([1, d.BS], np.float32),
                            np.zeros([1, d.BS], np.float32)])
    rzero = np.zeros([128, 512], np.float32)
    in_maps = []
    for c in range(N_CORES):
        wq_c = np.ascontiguousarray(
            Wq_f[:, c * d.QCH:(c + 1) * d.QCH]).astype(bf16)
        wkv_c = np.ascontiguousarray(np.concatenate(
            [Wk_f[:, c * d.HD:(c + 1) * d.HD],
             Wv_f[:, c * d.HD:(c + 1) * d.HD]], axis=1)).astype(bf16)
        wo_c = np.ascontiguousarray(
            Wo_f[c * d.QCH:(c + 1) * d.QCH, :]).astype(bf16)
        in_maps.append({
            "hT": hT, "wq": wq_c, "wkv": wkv_c, "wo": wo_c,
            "emat": emat, "id66": id66, "vones": vones,
            "rzero": rzero,
        })
    return in_maps


def kernel(hidden_states, Wq, Wk, Wv, Wo, cos, sin):
    d = Dims()
    nc = _get_program()
    in_maps = make_in_maps(hidden_states, Wq, Wk, Wv, Wo, cos, sin, d)
    res = bass_utils.run_bass_kernel_spmd(
        nc, in_maps, core_ids=list(range(N_CORES)))
    acc = res.results[0]["out"].astype(np.float32)
    for c in range(1, N_CORES):
        acc += res.results[c]["out"].astype(np.float32)
    return acc.reshape(B, S, HID)


if __name__ == "__main__":
    rng = np.random.default_rng(0)
    h = rng.standard_normal((B, S, HID), dtype=np.float32)
    sc = 1.0 / np.sqrt(HID)
    Wq_ = rng.standard_normal((HID, NH * HD), dtype=np.float32) * sc
    Wk_ = rng.standard_normal((HID, KVH * HD), dtype=np.float32) * sc
    Wv_ = rng.standard_normal((HID, KVH * HD), dtype=np.float32) * sc
    Wo_ = rng.standard_normal((NH * HD, HID), dtype=np.float32) * sc
    inv = 1.0 / (10000.0 ** (np.arange(0, HD, 2, dtype=np.float32) / HD))
    t = np.arange(S, dtype=np.float32)
    fr = np.outer(t, inv)
    emb = np.concatenate([fr, fr], axis=-1)
    o = kernel(h, Wq_, Wk_, Wv_, Wo_, np.cos(emb), np.sin(emb))
    print("out", o.shape, o.dtype, float(np.abs(o).max()))
